# revision 14
# baseline (speedup 1.0000x reference)
"""Trainium2 Bass kernel for nn_KANCouplingNet (3-layer KAN MLP, widths 12-64-64-24).

Math: each KAN layer is y = silu(x) @ sb + B(x) contracted with coef*ss.  On
the uniform grid the basis is the cardinal cubic B-spline
    6*M(w) = relu(2-w)^3 - 4*relu(1-w)^3,   w = |s - g'|,  s = x/0.4 + 5.5.
The kernel computes the merged basis value directly (ONE feature per basis
function -> half the matmul contraction rows of the two-cube scheme) and
balances the per-page elementwise work across the Scalar and Vector engines:

  'W' pages:  Scalar: w = Abs(ps + bias_p);  u = Relu(2 - w)
              Vector: f = u^3 - 4*relu(u-1)^3          (custom op, one pass,
              batched over all W pages of a layer in a single instruction)
  'P' pages:  Vector: r2 = relu(2-|ps-c0|)^3 (paged over all P pages), then
              f = r2 - 4*relu(1-min(|ps-c0|,1))^3 per page.

Layer-0/1 stationary weights are pre-scaled by 2.5 so PSUM holds s-space
(the 5.5 shift is absorbed into the per-partition offsets); silu reads the
same PSUM with scale=0.4.  Layer 0 works in x-space directly (clamps 0.8/0.4,
weights absorb 2.5^3) on an 8-fold broadcast of x done by one DMA.

Sharding: pure data parallel over the batch dim (32 batches -> 4 per core).
"""
import dataclasses

import numpy as np

import concourse.bacc as bacc
import concourse.bass as bass
import concourse.mybir as mybir
import concourse.tile as tile
from concourse.bass_utils import run_bass_kernel_spmd

FP = mybir.dt.float32
FPR = mybir.dt.float32r
FP16 = mybir.dt.float16
AFT = mybir.ActivationFunctionType

N_CORES = 8
B_PER_CORE = 4          # 32 batches / 8 cores
HW = 64 * 64            # 4096 pixels per batch image
NT = 1024               # pixel tile
TILES_PER_B = HW // NT  # 4
H_GRID = 0.4
S_SCALE = 1.0 / H_GRID          # 2.5
S_BIAS = 2.2 / H_GRID           # 5.5
WIDTH = [12, 64, 64, 24]

# number of scalar-assisted ('W') pages per layer (first nW pages); rest 'P'
N_W = {"L1": 2, "L2": 3}

_OPS = {}
_CACHE = {}


def _register_ops():
    """Append the custom DVE ops to dve_ops.OPS (idempotent, fixed order)."""
    if _OPS:
        return _OPS
    from concourse import dve_ops
    from concourse.dve_spec import (AluOp, Bin, C0, C1, C2, One, PageIdx, Spec,
                                    Src0, Src1, Zero, _has_src1, lower, minn,
                                    relu, sq)
    from concourse.dve_uop import DveOpSpec

    def mk(name, body, ref, subdim):
        for op in dve_ops.OPS:
            if op.name == name:
                _OPS[name] = op
                return
        spec = Spec(body=body, reference=ref)
        row = dve_ops._CUSTOM_DVE_ROW_BASE + len(dve_ops.OPS)
        rd1 = _has_src1(spec)
        shas = {}
        for ver in ("v3", "v4"):
            tmp = DveOpSpec(name=name, opcode=row,
                            uops=lower(spec, ver=ver), rd1_en=rd1)
            shas[ver] = tmp.sha(ver)
        op = dve_ops.DveOp(name, spec, subdim=subdim, uops_sha=shas)
        dve_ops.OPS.append(op)
        dve_ops._SUB_OPCODE_FOR_NAME[name] = row
        dve_ops.CUSTOM_DVE_SPECS[name] = spec
        _OPS[name] = op

    # paged cube: out = relu(imm2 - |in0 - (s0 + page*s1)|)^3
    pg = PageIdx(C0, C1)
    w_pg = Bin(AluOp.ABSOLUTE_DIFF, Src0, pg)
    r_pg = relu(Bin(AluOp.SUBTRACT, C2, w_pg))

    def _ref_cube(in0, in1, s0, s1, imm2):
        in0 = np.asarray(in0, np.float32)
        if in0.ndim == 3:
            pgv = np.asarray(s0).reshape(-1, 1, 1) + np.arange(in0.shape[1]).reshape(1, -1, 1) * s1
        else:
            pgv = np.asarray(s0).reshape(-1, 1)
        r = np.maximum(imm2 - np.abs(in0 - pgv), 0.0).astype(np.float32)
        return r * r * r
    mk("CUBE_FOLD_ANT", sq(r_pg) * r_pg, _ref_cube, True)

    # W-page finish: in0 = u = relu(2-w); out = u^3 - s0*relu(u-1)^3
    u = Src0
    s1p = u - One
    R = relu(s1p)
    body_pos = (sq(u) * u) - ((s1p * R) * R) * C0

    def _ref_pos(in0, in1, s0, s1, imm2):
        u = np.asarray(in0, np.float32)
        s1p = u - 1.0
        R = np.maximum(s1p, 0.0)
        return (u * u * u - s0 * ((s1p * R) * R)).astype(np.float32)
    mk("MERGE_POS_ANT", body_pos, _ref_pos, False)

    # P-page finish: out = in1 - s1*(imm2 - min(|in0 - s0|, imm2))^3
    w = Bin(AluOp.ABSOLUTE_DIFF, Src0, C0)
    b = C2 - minn(w, C2)
    body_flat = Src1 - (sq(b) * b) * C1

    def _ref_flat(in0, in1, s0, s1, imm2):
        w = np.abs(np.asarray(in0, np.float32) - np.asarray(s0).reshape(-1, 1))
        b = imm2 - np.minimum(w, imm2)
        return (np.asarray(in1, np.float32) - s1 * b * b * b).astype(np.float32)
    mk("MERGE_FLATX_ANT", body_flat, _ref_flat, False)

    return _OPS


def _paged(ap: bass.AP, s: int) -> bass.AP:
    """View a flat [P, N] AP as [P, s, N] with a step-0 page dim."""
    return dataclasses.replace(ap, ap=[ap.ap[0], [0, s], ap.ap[1]])


def _rep8(src: bass.AP) -> bass.AP:
    """Prepend a stride-0 x8 replication dim to a DRAM [12, N] AP."""
    return dataclasses.replace(src, ap=[[0, 8]] + src.ap)


def _host_weights(coef, sb, ss, li):
    din, dout = WIDTH[li], WIDTH[li + 1]
    cp = coef.astype(np.float64) * ss.astype(np.float64)[:, :, None] / 6.0
    pre = S_SCALE if li < 2 else 1.0
    mcols = 128 if dout == 64 else dout
    if li == 0:
        scale = pre * S_SCALE ** 3       # x-space cubes absorb 2.5^3
        w = np.zeros((108, mcols), np.float32)
        for g in range(8):
            for i in range(12):
                for o in range(dout):
                    v = scale * cp[i, o, g]
                    w[g * 12 + i, o] = v
                    if mcols == 128:
                        w[g * 12 + i, o + 64] = v
        for i in range(12):
            for o in range(dout):
                w[96 + i, o] = pre * sb[i, o]
                if mcols == 128:
                    w[96 + i, o + 64] = pre * sb[i, o]
        return [w], None
    pages = []
    for j in range(4):
        w = np.zeros((128, mcols), np.float32)
        for p in range(128):
            i, g = p % 64, 4 * (p // 64) + j
            for o in range(dout):
                v = pre * cp[i, o, g]
                w[p, o] = v
                if mcols == 128:
                    w[p, o + 64] = v
        pages.append(w)
    base = np.zeros((64, mcols), np.float32)
    base[:, :dout] = pre * sb
    if mcols == 128:
        base[:, 64:64 + dout] = pre * sb
    return pages, base


def _build():
    ops = _register_ops()
    CUBE, MPOS, MFLAT = (ops["CUBE_FOLD_ANT"], ops["MERGE_POS_ANT"],
                         ops["MERGE_FLATX_ANT"])
    nc = bacc.Bacc("TRN2", target_bir_lowering=False, debug=False,
                   enable_asserts=False, num_devices=N_CORES)

    x_d = nc.dram_tensor("x_in", [B_PER_CORE, 12, HW], FP, kind="ExternalInput").ap()
    out_d = nc.dram_tensor("y_out", [B_PER_CORE, 24, HW], FP, kind="ExternalOutput").ap()
    w0_d = nc.dram_tensor("w0", [108, 128], FPR, kind="ExternalInput").ap()
    w1_d = nc.dram_tensor("w1", [4, 128, 128], FPR, kind="ExternalInput").ap()
    b1_d = nc.dram_tensor("b1", [64, 128], FPR, kind="ExternalInput").ap()
    w2_d = nc.dram_tensor("w2", [4, 128, 24], FPR, kind="ExternalInput").ap()
    b2_d = nc.dram_tensor("b2", [64, 24], FPR, kind="ExternalInput").ap()
    c0x_d = nc.dram_tensor("c0x", [96, 1], FP, kind="ExternalInput").ap()
    bias_d = nc.dram_tensor("biasT", [128, 4], FP, kind="ExternalInput").ap()
    c0t_d = nc.dram_tensor("c0T", [128, 4], FP, kind="ExternalInput").ap()

    nW1, nW2 = N_W["L1"], N_W["L2"]

    with tile.TileContext(nc) as tc:
        with (
            tc.tile_pool(name="consts", bufs=1) as cp,
            tc.tile_pool(name="xrep", bufs=3) as xp,
            tc.tile_pool(name="wabs", bufs=3) as wp,
            tc.tile_pool(name="ucl", bufs=2) as up,
            tc.tile_pool(name="r2c", bufs=2) as rp,
            tc.tile_pool(name="f0", bufs=3) as f0p,
            tc.tile_pool(name="f1", bufs=2) as f1p,
            tc.tile_pool(name="f2", bufs=2) as f2p,
            tc.tile_pool(name="sil", bufs=2) as silp,
            tc.tile_pool(name="ps1", bufs=1, space="PSUM") as pp1,
            tc.tile_pool(name="ps2", bufs=2, space="PSUM") as pp2,
            tc.tile_pool(name="ps3", bufs=2, space="PSUM") as pp3,
        ):
            # ---- constants (small per-partition vectors FIRST so the
            # feature engines can start while the weights stream in) ----
            c0x = cp.tile([96, 1], FP, tag="c0x")
            nc.sync.dma_start(c0x[:], c0x_d[:])
            biasT = cp.tile([128, 4], FP, tag="biasT")
            nc.sync.dma_start(biasT[:], bias_d[:])
            c0T = cp.tile([128, 4], FP, tag="c0T")
            nc.sync.dma_start(c0T[:], c0t_d[:])
            bias2 = cp.tile([128, 1], FP, tag="bias2")
            nc.gpsimd.memset(bias2[:], 2.0)
            w0 = cp.tile([108, 128], FPR, tag="w0")
            nc.sync.dma_start(w0[:], w0_d[:])
            b1 = cp.tile([64, 128], FPR, tag="b1")
            nc.sync.dma_start(b1[:], b1_d[:])
            b2 = cp.tile([64, 24], FPR, tag="b2")
            nc.sync.dma_start(b2[:], b2_d[:])
            w1 = [cp.tile([128, 128], FPR, tag=f"w1_{j}", name=f"w1_{j}") for j in range(4)]
            w2 = [cp.tile([128, 24], FPR, tag=f"w2_{j}", name=f"w2_{j}") for j in range(4)]
            for j in range(4):
                nc.sync.dma_start(w1[j][:], w1_d[j])
                nc.sync.dma_start(w2[j][:], w2_d[j])

            def mm(ps, w_t, f_t, col0, start, stop):
                for h in range(2):
                    nc.tensor.matmul(ps[:, h * 512:(h + 1) * 512], w_t[:],
                                     f_t[:, col0 + h * 512:col0 + (h + 1) * 512],
                                     start=start, stop=stop)

            def feat_pages(ps, f_tile, nW, u_tag, layer):
                # W pages: scalar abs+relu into u slices, one batched DVE finish
                if nW:
                    u = up.tile([128, nW * NT], FP, tag=u_tag, name=f"{u_tag}")
                    for j in range(nW):
                        wt = wp.tile([128, NT], FP, tag="wt")
                        nc.scalar.activation(wt[:], ps[:], AFT.Abs,
                                             bias=biasT[:, j:j + 1], scale=1.0)
                        nc.scalar.activation(u[:, bass.ts(j, NT)], wt[:],
                                             AFT.Relu, bias=bias2[:], scale=-1.0)
                    nc.vector._custom_dve(MPOS, out=f_tile[:, 0:nW * NT],
                                          in0=u[:], s0=4.0)
                # P pages: one paged cube + per-page flat merge
                nP = 4 - nW
                if nP:
                    r2 = rp.tile([128, nP * NT], FP, tag=f"r2_{layer}",
                                 name=f"r2_{layer}")
                    nc.vector._custom_dve(
                        CUBE, out=dataclasses.replace(
                            r2[:], ap=[r2[:].ap[0], [NT, nP], [1, NT]]),
                        in0=_paged(ps[:], nP),
                        s0=c0T[:, nW:nW + 1], s1=1.0, imm2=2.0)
                    for k in range(nP):
                        nc.vector._custom_dve(
                            MFLAT, out=f_tile[:, bass.ts(nW + k, NT)],
                            in0=ps[:], in1=r2[:, bass.ts(k, NT)],
                            s0=c0T[:, nW + k:nW + k + 1], s1=4.0, imm2=1.0)

            T = B_PER_CORE * TILES_PER_B
            st = {}

            def cols_of(t):
                b, ti = divmod(t, TILES_PER_B)
                return b, bass.ts(ti, NT)

            def dma_in(t):
                b, cols = cols_of(t)
                s0r = xp.tile([96, NT], FP, tag="s0r")
                nc.gpsimd.dma_start(s0r[:], _rep8(x_d[b, :, cols]))
                st[("s0r", t)] = s0r

            def l0_feats(t):
                s0r = st[("s0r", t)]
                f0 = f0p.tile([108, NT], FPR, tag="f0")
                r20 = rp.tile([96, NT], FP, tag="r20")
                nc.vector._custom_dve(CUBE, out=_paged(r20[:], 1),
                                      in0=_paged(s0r[:], 1),
                                      s0=c0x[:], s1=0.0, imm2=2 * H_GRID)
                nc.vector._custom_dve(MFLAT, out=f0[0:96, :], in0=s0r[:],
                                      in1=r20[:], s0=c0x[:], s1=4.0,
                                      imm2=H_GRID)
                st[("f0", t)] = f0

            def acts(t, ps, nW, u_tag):
                u = up.tile([128, nW * NT], FP, tag=u_tag, name=u_tag) if nW else None
                for j in range(nW):
                    wt = wp.tile([128, NT], FP, tag="wt")
                    nc.scalar.activation(wt[:], ps[:], AFT.Abs,
                                         bias=biasT[:, j:j + 1], scale=1.0)
                    nc.scalar.activation(u[:, bass.ts(j, NT)], wt[:],
                                         AFT.Relu, bias=bias2[:], scale=-1.0)
                return u

            def pcube(t, ps, f_tile, nW, layer):
                nP = 4 - nW
                if not nP:
                    return
                r2 = rp.tile([128, nP * NT], FP, tag=f"r2_{layer}",
                             name=f"r2_{layer}")
                nc.vector._custom_dve(
                    CUBE, out=dataclasses.replace(
                        r2[:], ap=[r2[:].ap[0], [NT, nP], [1, NT]]),
                    in0=_paged(ps[:], nP),
                    s0=c0T[:, nW:nW + 1], s1=1.0, imm2=2.0)
                for k in range(nP):
                    nc.vector._custom_dve(
                        MFLAT, out=f_tile[:, bass.ts(nW + k, NT)],
                        in0=ps[:], in1=r2[:, bass.ts(k, NT)],
                        s0=c0T[:, nW + k:nW + k + 1], s1=4.0, imm2=1.0)

            def mpos(u, f_tile, nW):
                if nW:
                    nc.vector._custom_dve(MPOS, out=f_tile[:, 0:nW * NT],
                                          in0=u[:], s0=4.0)

            dma_in(0)
            if T > 1:
                dma_in(1)
            if T > 2:
                dma_in(2)
            for t0_ in range(min(3, T)):
                l0_feats(t0_)
                s0r_ = st.pop(("s0r", t0_))
                f0_ = st[("f0", t0_)]
                nc.scalar.activation(f0_[96:108, :], s0r_[0:12, :], AFT.Silu)
            f0_ = st.pop(("f0", 0))
            ps1_ = pp1.tile([128, NT], FP, tag="ps1")
            st[("ps1", 0)] = ps1_
            mm(ps1_, w0, f0_, 0, True, True)
            for j in range(-1, T + 2):
                if 3 <= j + 2 < T:
                    dma_in(j + 2)
                # vector: L2 P-pages of tile j-1 (ps2 ready since last iter)
                if 0 <= j - 1 < T:
                    ps2 = st[("ps2", j - 1)]
                    f2 = f2p.tile([128, 4 * NT], FPR, tag="f2")
                    st[("f2", j - 1)] = f2
                    pcube(j - 1, ps2, f2, nW2, 2)
                # vector: L0 features of tile j+1 (tiles 0-2 done in prologue)
                if 3 <= j + 1 < T:
                    l0_feats(j + 1)
                # scalar: L1 activations of tile j
                if 0 <= j < T:
                    ps1 = st[("ps1", j)]
                    f1 = f1p.tile([128, 4 * NT], FPR, tag="f1")
                    st[("f1", j)] = f1
                    u1 = acts(j, ps1, nW1, "u1")
                    pcube(j, ps1, f1, nW1, 1)
                    mpos(u1, f1, nW1)
                    st[("ps1r", j)] = ps1
                # scalar: L2 activations of tile j-1; vector: L2 W finish
                if 0 <= j - 1 < T:
                    ps2 = st.pop(("ps2", j - 1))
                    u2 = acts(j - 1, ps2, nW2, "u2")
                    f2 = st.pop(("f2", j - 1))
                    mpos(u2, f2, nW2)
                    sil2 = silp.tile([64, NT], FPR, tag="sil2")
                    nc.scalar.activation(sil2[:], ps2[0:64, :], AFT.Silu,
                                         scale=H_GRID)
                    # PE: ps3 halves + yt + out
                    halves = []
                    for h in range(2):
                        ps3 = pp3.tile([24, 512], FP, tag="ps3")
                        for k in range(4):
                            nc.tensor.matmul(ps3[:], w2[k][:],
                                             f2[:, k * NT + h * 512:k * NT + (h + 1) * 512],
                                             start=k == 0, stop=False)
                        nc.tensor.matmul(ps3[:], b2[:],
                                         sil2[:, h * 512:(h + 1) * 512],
                                         start=False, stop=True)
                        halves.append(ps3)
                    st[("ps3h", j - 1)] = halves
                # scalar tail: sil1 of tile j (consumed only by the PE late),
                # L0 silu of tile j+1, output copies of tile j-2
                if 0 <= j < T:
                    ps1r = st.pop(("ps1r", j))
                    sil1 = silp.tile([64, NT], FPR, tag="sil1")
                    nc.scalar.activation(sil1[:], ps1r[0:64, :], AFT.Silu,
                                         scale=H_GRID)
                    st[("sil1", j)] = sil1
                if 3 <= j + 1 < T:
                    s0r = st.pop(("s0r", j + 1))
                    f0 = st[("f0", j + 1)]
                    nc.scalar.activation(f0[96:108, :], s0r[0:12, :], AFT.Silu)
                if ("ps3h", j - 2) in st:
                    b2_, ti2 = divmod(j - 2, TILES_PER_B)
                    for h, ps3h in enumerate(st.pop(("ps3h", j - 2))):
                        yt = silp.tile([24, 512], FP, tag="yt")
                        nc.scalar.activation(yt[:], ps3h[:], AFT.Identity)
                        nc.sync.dma_start(
                            out_d[b2_, :, ti2 * NT + h * 512:ti2 * NT + (h + 1) * 512],
                            yt[:])
                # PE: ps2 of tile j
                if 0 <= j < T:
                    f1 = st.pop(("f1", j))
                    sil1 = st.pop(("sil1", j))
                    ps2 = pp2.tile([128, NT], FP, tag="ps2")
                    st[("ps2", j)] = ps2
                    for k in range(4):
                        mm(ps2, w1[k], f1, k * NT, k == 0, False)
                    mm(ps2, b1, sil1, 0, False, True)
                # PE: ps1 of tile j+1 (tile 0 done in prologue)
                if 0 < j + 1 < T:
                    f0 = st.pop(("f0", j + 1))
                    ps1 = pp1.tile([128, NT], FP, tag="ps1")
                    st[("ps1", j + 1)] = ps1
                    mm(ps1, w0, f0, 0, True, True)

    nc.compile()
    return nc


def _host_consts(coef0, sb0, ss0, coef1, sb1, ss1, coef2, sb2, ss2):
    w0, _ = _host_weights(coef0, sb0, ss0, 0)
    w1, b1 = _host_weights(coef1, sb1, ss1, 1)
    w2, b2 = _host_weights(coef2, sb2, ss2, 2)
    gp = np.arange(96) // 12
    c0x = (((2.0 + gp) - S_BIAS) * H_GRID).astype(np.float32).reshape(96, 1)
    p = np.arange(128)
    bias = np.zeros((128, 4), np.float32)
    for j in range(4):
        gidx = 4 * (p // 64) + j
        bias[:, j] = S_BIAS - (2.0 + gidx)
    c0t = (-bias).astype(np.float32)
    return {
        "w0": w0[0], "w1": np.stack(w1), "b1": b1,
        "w2": np.stack(w2), "b2": b2, "c0x": c0x,
        "biasT": bias, "c0T": c0t,
    }


def _in_maps(x):
    consts = _CACHE["consts"]
    x = np.asarray(x, np.float32).reshape(32, 12, HW)
    maps = []
    for c in range(N_CORES):
        m = dict(consts)
        m["x_in"] = np.ascontiguousarray(x[c * B_PER_CORE:(c + 1) * B_PER_CORE])
        maps.append(m)
    return maps


def kernel(x, grid0, coef0, sb0, ss0, grid1, coef1, sb1, ss1, grid2, coef2, sb2, ss2):
    if "nc" not in _CACHE:
        _CACHE["nc"] = _build()
    nc = _CACHE["nc"]
    _CACHE["consts"] = _host_consts(
        np.asarray(coef0, np.float32), np.asarray(sb0, np.float32), np.asarray(ss0, np.float32),
        np.asarray(coef1, np.float32), np.asarray(sb1, np.float32), np.asarray(ss1, np.float32),
        np.asarray(coef2, np.float32), np.asarray(sb2, np.float32), np.asarray(ss2, np.float32))
    maps = _in_maps(x)
    res = run_bass_kernel_spmd(nc, maps, core_ids=list(range(N_CORES)))
    _CACHE["maps"] = maps
    out = np.empty((32, 24, HW), np.float32)
    for c in range(N_CORES):
        out[c * B_PER_CORE:(c + 1) * B_PER_CORE] = res.results[c]["y_out"]
    return out.reshape(32, 24, 64, 64)


def _install_ntff_hook():
    import sys, types
    if "antenv.axon_hooks" in sys.modules:
        return
    state = {"hook": None}
    mod = types.ModuleType("antenv.axon_hooks")
    mod.set_axon_ntff_profile_hook = lambda h: state.__setitem__("hook", h)
    mod.get_axon_ntff_profile_hook = lambda: state["hook"]
    sys.modules["antenv.axon_hooks"] = mod
    import antenv
    antenv.axon_hooks = mod
    from trn_agent_boot.trn_boot import _ntff_profile_via_ctypes
    hook = _ntff_profile_via_ctypes("/opt/axon/libaxon_pjrt.so")
    if hook is not None:
        mod.set_axon_ntff_profile_hook(hook)


def profile():
    _install_ntff_hook()
    nc = _CACHE["nc"]
    res = run_bass_kernel_spmd(nc, _CACHE["maps"], core_ids=list(range(N_CORES)),
                               trace=True)
    return res.exec_time_ns, getattr(res, "instructions_and_trace", None)


# revision 15
# speedup vs baseline: 1.0380x; 1.0380x over previous
"""Trainium2 Bass kernel for nn_KANCouplingNet (3-layer KAN MLP, widths 12-64-64-24).

Math: each KAN layer is y = silu(x) @ sb + B(x) contracted with coef*ss.  On
the uniform grid the basis is the cardinal cubic B-spline
    6*M(w) = relu(2-w)^3 - 4*relu(1-w)^3,   w = |s - g'|,  s = x/0.4 + 5.5.
The kernel computes the merged basis value directly (ONE feature per basis
function -> half the matmul contraction rows of the two-cube scheme) and
balances the per-page elementwise work across the Scalar and Vector engines:

  'W' pages:  Scalar: w = Abs(ps + bias_p);  u = Relu(2 - w)
              Vector: f = u^3 - 4*relu(u-1)^3          (custom op, one pass,
              batched over all W pages of a layer in a single instruction)
  'P' pages:  Vector: r2 = relu(2-|ps-c0|)^3 (paged over all P pages), then
              f = r2 - 4*relu(1-min(|ps-c0|,1))^3 per page.

Layer-0/1 stationary weights are pre-scaled by 2.5 so PSUM holds s-space
(the 5.5 shift is absorbed into the per-partition offsets); silu reads the
same PSUM with scale=0.4.  Layer 0 works in x-space directly (clamps 0.8/0.4,
weights absorb 2.5^3) on an 8-fold broadcast of x done by one DMA.

Sharding: pure data parallel over the batch dim (32 batches -> 4 per core).
"""
import dataclasses

import numpy as np

import concourse.bacc as bacc
import concourse.bass as bass
import concourse.mybir as mybir
import concourse.tile as tile
from concourse.bass_utils import run_bass_kernel_spmd

FP = mybir.dt.float32
FPR = mybir.dt.float32r
FP16 = mybir.dt.float16
AFT = mybir.ActivationFunctionType

N_CORES = 8
B_PER_CORE = 4          # 32 batches / 8 cores
HW = 64 * 64            # 4096 pixels per batch image
NT = 1024               # pixel tile
TILES_PER_B = HW // NT  # 4
H_GRID = 0.4
S_SCALE = 1.0 / H_GRID          # 2.5
S_BIAS = 2.2 / H_GRID           # 5.5
WIDTH = [12, 64, 64, 24]

# number of scalar-assisted ('W') pages per layer (first nW pages); rest 'P'
N_W = {"L1": 2, "L2": 3}

_OPS = {}
_CACHE = {}


def _register_ops():
    """Append the custom DVE ops to dve_ops.OPS (idempotent, fixed order)."""
    if _OPS:
        return _OPS
    from concourse import dve_ops
    from concourse.dve_spec import (AluOp, Bin, C0, C1, C2, One, PageIdx, Spec,
                                    Src0, Src1, Zero, _has_src1, lower, minn,
                                    relu, sq)
    from concourse.dve_uop import DveOpSpec

    def mk(name, body, ref, subdim):
        for op in dve_ops.OPS:
            if op.name == name:
                _OPS[name] = op
                return
        spec = Spec(body=body, reference=ref)
        row = dve_ops._CUSTOM_DVE_ROW_BASE + len(dve_ops.OPS)
        rd1 = _has_src1(spec)
        shas = {}
        for ver in ("v3", "v4"):
            tmp = DveOpSpec(name=name, opcode=row,
                            uops=lower(spec, ver=ver), rd1_en=rd1)
            shas[ver] = tmp.sha(ver)
        op = dve_ops.DveOp(name, spec, subdim=subdim, uops_sha=shas)
        dve_ops.OPS.append(op)
        dve_ops._SUB_OPCODE_FOR_NAME[name] = row
        dve_ops.CUSTOM_DVE_SPECS[name] = spec
        _OPS[name] = op

    # paged cube: out = relu(imm2 - |in0 - (s0 + page*s1)|)^3
    pg = PageIdx(C0, C1)
    w_pg = Bin(AluOp.ABSOLUTE_DIFF, Src0, pg)
    r_pg = relu(Bin(AluOp.SUBTRACT, C2, w_pg))

    def _ref_cube(in0, in1, s0, s1, imm2):
        in0 = np.asarray(in0, np.float32)
        if in0.ndim == 3:
            pgv = np.asarray(s0).reshape(-1, 1, 1) + np.arange(in0.shape[1]).reshape(1, -1, 1) * s1
        else:
            pgv = np.asarray(s0).reshape(-1, 1)
        r = np.maximum(imm2 - np.abs(in0 - pgv), 0.0).astype(np.float32)
        return r * r * r
    mk("CUBE_FOLD_ANT", sq(r_pg) * r_pg, _ref_cube, True)

    # W-page finish: in0 = u = relu(2-w); out = u^3 - s0*relu(u-1)^3
    u = Src0
    s1p = u - One
    R = relu(s1p)
    body_pos = (sq(u) * u) - ((s1p * R) * R) * C0

    def _ref_pos(in0, in1, s0, s1, imm2):
        u = np.asarray(in0, np.float32)
        s1p = u - 1.0
        R = np.maximum(s1p, 0.0)
        return (u * u * u - s0 * ((s1p * R) * R)).astype(np.float32)
    mk("MERGE_POS_ANT", body_pos, _ref_pos, False)

    # P-page finish: out = in1 - s1*(imm2 - min(|in0 - s0|, imm2))^3
    w = Bin(AluOp.ABSOLUTE_DIFF, Src0, C0)
    b = C2 - minn(w, C2)
    body_flat = Src1 - (sq(b) * b) * C1

    def _ref_flat(in0, in1, s0, s1, imm2):
        w = np.abs(np.asarray(in0, np.float32) - np.asarray(s0).reshape(-1, 1))
        b = imm2 - np.minimum(w, imm2)
        return (np.asarray(in1, np.float32) - s1 * b * b * b).astype(np.float32)
    mk("MERGE_FLATX_ANT", body_flat, _ref_flat, False)

    return _OPS


def _paged(ap: bass.AP, s: int) -> bass.AP:
    """View a flat [P, N] AP as [P, s, N] with a step-0 page dim."""
    return dataclasses.replace(ap, ap=[ap.ap[0], [0, s], ap.ap[1]])


def _rep8(src: bass.AP) -> bass.AP:
    """Prepend a stride-0 x8 replication dim to a DRAM [12, N] AP."""
    return dataclasses.replace(src, ap=[[0, 8]] + src.ap)


def _host_weights(coef, sb, ss, li):
    din, dout = WIDTH[li], WIDTH[li + 1]
    cp = coef.astype(np.float64) * ss.astype(np.float64)[:, :, None] / 6.0
    pre = S_SCALE if li < 2 else 1.0
    mcols = 128 if dout == 64 else dout
    if li == 0:
        scale = pre * S_SCALE ** 3       # x-space cubes absorb 2.5^3
        w = np.zeros((108, mcols), np.float32)
        for g in range(8):
            for i in range(12):
                for o in range(dout):
                    v = scale * cp[i, o, g]
                    w[g * 12 + i, o] = v
                    if mcols == 128:
                        w[g * 12 + i, o + 64] = v
        for i in range(12):
            for o in range(dout):
                w[96 + i, o] = pre * sb[i, o]
                if mcols == 128:
                    w[96 + i, o + 64] = pre * sb[i, o]
        return [w], None
    pages = []
    for j in range(4):
        w = np.zeros((128, mcols), np.float32)
        for p in range(128):
            i, g = p % 64, 4 * (p // 64) + j
            for o in range(dout):
                v = pre * cp[i, o, g]
                w[p, o] = v
                if mcols == 128:
                    w[p, o + 64] = v
        pages.append(w)
    base = np.zeros((64, mcols), np.float32)
    base[:, :dout] = pre * sb
    if mcols == 128:
        base[:, 64:64 + dout] = pre * sb
    return pages, base


def _build():
    ops = _register_ops()
    CUBE, MPOS, MFLAT = (ops["CUBE_FOLD_ANT"], ops["MERGE_POS_ANT"],
                         ops["MERGE_FLATX_ANT"])
    nc = bacc.Bacc("TRN2", target_bir_lowering=False, debug=False,
                   enable_asserts=False, num_devices=N_CORES)

    x_d = nc.dram_tensor("x_in", [B_PER_CORE, 12, HW], FP, kind="ExternalInput").ap()
    out_d = nc.dram_tensor("y_out", [B_PER_CORE, 24, HW], FP, kind="ExternalOutput").ap()
    w0_d = nc.dram_tensor("w0", [108, 128], FPR, kind="ExternalInput").ap()
    w1_d = nc.dram_tensor("w1", [4, 128, 128], FPR, kind="ExternalInput").ap()
    b1_d = nc.dram_tensor("b1", [64, 128], FPR, kind="ExternalInput").ap()
    w2_d = nc.dram_tensor("w2", [4, 128, 24], FPR, kind="ExternalInput").ap()
    b2_d = nc.dram_tensor("b2", [64, 24], FPR, kind="ExternalInput").ap()
    c0x_d = nc.dram_tensor("c0x", [96, 1], FP, kind="ExternalInput").ap()
    bias_d = nc.dram_tensor("biasT", [128, 4], FP, kind="ExternalInput").ap()
    c0t_d = nc.dram_tensor("c0T", [128, 4], FP, kind="ExternalInput").ap()

    nW1, nW2 = N_W["L1"], N_W["L2"]

    with tile.TileContext(nc) as tc:
        with (
            tc.tile_pool(name="consts", bufs=1) as cp,
            tc.tile_pool(name="xrep", bufs=3) as xp,
            tc.tile_pool(name="wabs", bufs=3) as wp,
            tc.tile_pool(name="ucl", bufs=2) as up,
            tc.tile_pool(name="r2c", bufs=2) as rp,
            tc.tile_pool(name="f0", bufs=3) as f0p,
            tc.tile_pool(name="f1", bufs=2) as f1p,
            tc.tile_pool(name="f2", bufs=2) as f2p,
            tc.tile_pool(name="sil", bufs=2) as silp,
            tc.tile_pool(name="ps1", bufs=1, space="PSUM") as pp1,
            tc.tile_pool(name="ps2", bufs=2, space="PSUM") as pp2,
            tc.tile_pool(name="ps3", bufs=2, space="PSUM") as pp3,
        ):
            # ---- constants (small per-partition vectors FIRST so the
            # feature engines can start while the weights stream in) ----
            c0x = cp.tile([96, 1], FP, tag="c0x")
            nc.sync.dma_start(c0x[:], c0x_d[:])
            biasT = cp.tile([128, 4], FP, tag="biasT")
            nc.sync.dma_start(biasT[:], bias_d[:])
            c0T = cp.tile([128, 4], FP, tag="c0T")
            nc.sync.dma_start(c0T[:], c0t_d[:])
            bias2 = cp.tile([128, 1], FP, tag="bias2")
            nc.gpsimd.memset(bias2[:], 2.0)
            w0 = cp.tile([108, 128], FPR, tag="w0")
            nc.sync.dma_start(w0[:], w0_d[:])
            b1 = cp.tile([64, 128], FPR, tag="b1")
            nc.sync.dma_start(b1[:], b1_d[:])
            b2 = cp.tile([64, 24], FPR, tag="b2")
            nc.sync.dma_start(b2[:], b2_d[:])
            w1 = [cp.tile([128, 128], FPR, tag=f"w1_{j}", name=f"w1_{j}") for j in range(4)]
            w2 = [cp.tile([128, 24], FPR, tag=f"w2_{j}", name=f"w2_{j}") for j in range(4)]
            for j in range(4):
                nc.sync.dma_start(w1[j][:], w1_d[j])
                nc.sync.dma_start(w2[j][:], w2_d[j])

            def mm(ps, w_t, f_t, col0, start, stop):
                for h in range(2):
                    nc.tensor.matmul(ps[:, h * 512:(h + 1) * 512], w_t[:],
                                     f_t[:, col0 + h * 512:col0 + (h + 1) * 512],
                                     start=start, stop=stop)

            def feat_pages(ps, f_tile, nW, u_tag, layer):
                # W pages: scalar abs+relu into u slices, one batched DVE finish
                if nW:
                    u = up.tile([128, nW * NT], FP, tag=u_tag, name=f"{u_tag}")
                    for j in range(nW):
                        wt = wp.tile([128, NT], FP, tag="wt")
                        nc.scalar.activation(wt[:], ps[:], AFT.Abs,
                                             bias=biasT[:, j:j + 1], scale=1.0)
                        nc.scalar.activation(u[:, bass.ts(j, NT)], wt[:],
                                             AFT.Relu, bias=bias2[:], scale=-1.0)
                    nc.vector._custom_dve(MPOS, out=f_tile[:, 0:nW * NT],
                                          in0=u[:], s0=4.0)
                # P pages: one paged cube + per-page flat merge
                nP = 4 - nW
                if nP:
                    r2 = rp.tile([128, nP * NT], FP, tag=f"r2_{layer}",
                                 name=f"r2_{layer}")
                    nc.vector._custom_dve(
                        CUBE, out=dataclasses.replace(
                            r2[:], ap=[r2[:].ap[0], [NT, nP], [1, NT]]),
                        in0=_paged(ps[:], nP),
                        s0=c0T[:, nW:nW + 1], s1=1.0, imm2=2.0)
                    for k in range(nP):
                        nc.vector._custom_dve(
                            MFLAT, out=f_tile[:, bass.ts(nW + k, NT)],
                            in0=ps[:], in1=r2[:, bass.ts(k, NT)],
                            s0=c0T[:, nW + k:nW + k + 1], s1=4.0, imm2=1.0)

            T = B_PER_CORE * TILES_PER_B
            st = {}

            def cols_of(t):
                b, ti = divmod(t, TILES_PER_B)
                return b, bass.ts(ti, NT)

            def dma_in(t):
                b, cols = cols_of(t)
                s0r = xp.tile([96, NT], FP, tag="s0r")
                nc.gpsimd.dma_start(s0r[:], _rep8(x_d[b, :, cols]))
                st[("s0r", t)] = s0r

            def l0_feats(t):
                s0r = st[("s0r", t)]
                f0 = f0p.tile([108, NT], FPR, tag="f0")
                r20 = rp.tile([96, NT], FP, tag="r20")
                nc.vector._custom_dve(CUBE, out=_paged(r20[:], 1),
                                      in0=_paged(s0r[:], 1),
                                      s0=c0x[:], s1=0.0, imm2=2 * H_GRID)
                nc.vector._custom_dve(MFLAT, out=f0[0:96, :], in0=s0r[:],
                                      in1=r20[:], s0=c0x[:], s1=4.0,
                                      imm2=H_GRID)
                st[("f0", t)] = f0

            def acts(t, ps, nW, u_tag):
                u = up.tile([128, nW * NT], FP, tag=u_tag, name=u_tag) if nW else None
                for j in range(nW):
                    wt = wp.tile([128, NT], FP, tag="wt")
                    nc.scalar.activation(wt[:], ps[:], AFT.Abs,
                                         bias=biasT[:, j:j + 1], scale=1.0)
                    nc.scalar.activation(u[:, bass.ts(j, NT)], wt[:],
                                         AFT.Relu, bias=bias2[:], scale=-1.0)
                return u

            def pcube(t, ps, f_tile, nW, layer):
                nP = 4 - nW
                if not nP:
                    return
                r2 = rp.tile([128, nP * NT], FP, tag=f"r2_{layer}",
                             name=f"r2_{layer}")
                nc.vector._custom_dve(
                    CUBE, out=dataclasses.replace(
                        r2[:], ap=[r2[:].ap[0], [NT, nP], [1, NT]]),
                    in0=_paged(ps[:], nP),
                    s0=c0T[:, nW:nW + 1], s1=1.0, imm2=2.0)
                for k in range(nP):
                    nc.vector._custom_dve(
                        MFLAT, out=f_tile[:, bass.ts(nW + k, NT)],
                        in0=ps[:], in1=r2[:, bass.ts(k, NT)],
                        s0=c0T[:, nW + k:nW + k + 1], s1=4.0, imm2=1.0)

            def mpos(u, f_tile, nW):
                if nW:
                    nc.vector._custom_dve(MPOS, out=f_tile[:, 0:nW * NT],
                                          in0=u[:], s0=4.0)

            dma_in(0)
            if T > 1:
                dma_in(1)
            if T > 2:
                dma_in(2)
            for t0_ in range(min(3, T)):
                l0_feats(t0_)
                s0r_ = st.pop(("s0r", t0_))
                f0_ = st[("f0", t0_)]
                nc.scalar.activation(f0_[96:108, :], s0r_[0:12, :], AFT.Silu)
            f0_ = st.pop(("f0", 0))
            ps1_ = pp1.tile([128, NT], FP, tag="ps1")
            st[("ps1", 0)] = ps1_
            mm(ps1_, w0, f0_, 0, True, True)
            for j in range(-1, T + 2):
                if 3 <= j + 2 < T:
                    dma_in(j + 2)
                # vector: L2 P-pages of tile j-1 (ps2 ready since last iter)
                if 0 <= j - 1 < T:
                    ps2 = st[("ps2", j - 1)]
                    f2 = f2p.tile([128, 4 * NT], FPR, tag="f2")
                    st[("f2", j - 1)] = f2
                    pcube(j - 1, ps2, f2, nW2, 2)
                # vector: L0 features of tile j+1 (tiles 0-2 done in prologue)
                if 3 <= j + 1 < T:
                    l0_feats(j + 1)
                # scalar: L1 activations of tile j
                if 0 <= j < T:
                    ps1 = st[("ps1", j)]
                    f1 = f1p.tile([128, 4 * NT], FPR, tag="f1")
                    st[("f1", j)] = f1
                    u1 = acts(j, ps1, nW1, "u1")
                    pcube(j, ps1, f1, nW1, 1)
                    mpos(u1, f1, nW1)
                    sil1 = silp.tile([64, NT], FPR, tag="sil1")
                    nc.scalar.activation(sil1[:], ps1[0:64, :], AFT.Silu,
                                         scale=H_GRID)
                    st[("sil1", j)] = sil1
                # scalar: L2 activations of tile j-1; vector: L2 W finish
                if 0 <= j - 1 < T:
                    ps2 = st.pop(("ps2", j - 1))
                    u2 = acts(j - 1, ps2, nW2, "u2")
                    f2 = st.pop(("f2", j - 1))
                    mpos(u2, f2, nW2)
                    sil2 = silp.tile([64, NT], FPR, tag="sil2")
                    nc.scalar.activation(sil2[:], ps2[0:64, :], AFT.Silu,
                                         scale=H_GRID)
                    # PE: ps3 halves + yt + out
                    halves = []
                    for h in range(2):
                        ps3 = pp3.tile([24, 512], FP, tag="ps3")
                        for k in range(4):
                            nc.tensor.matmul(ps3[:], w2[k][:],
                                             f2[:, k * NT + h * 512:k * NT + (h + 1) * 512],
                                             start=k == 0, stop=False)
                        nc.tensor.matmul(ps3[:], b2[:],
                                         sil2[:, h * 512:(h + 1) * 512],
                                         start=False, stop=True)
                        halves.append(ps3)
                    st[("ps3h", j - 1)] = halves
                # scalar tail: L0 silu of tile j+1, output copies of tile j-2
                if 3 <= j + 1 < T:
                    s0r = st.pop(("s0r", j + 1))
                    f0 = st[("f0", j + 1)]
                    nc.scalar.activation(f0[96:108, :], s0r[0:12, :], AFT.Silu)
                if ("ps3h", j - 2) in st:
                    b2_, ti2 = divmod(j - 2, TILES_PER_B)
                    for h, ps3h in enumerate(st.pop(("ps3h", j - 2))):
                        yt = silp.tile([24, 512], FP, tag="yt")
                        nc.scalar.activation(yt[:], ps3h[:], AFT.Identity)
                        nc.sync.dma_start(
                            out_d[b2_, :, ti2 * NT + h * 512:ti2 * NT + (h + 1) * 512],
                            yt[:])
                # PE: ps2 of tile j
                if 0 <= j < T:
                    f1 = st.pop(("f1", j))
                    sil1 = st.pop(("sil1", j))
                    ps2 = pp2.tile([128, NT], FP, tag="ps2")
                    st[("ps2", j)] = ps2
                    for k in range(4):
                        mm(ps2, w1[k], f1, k * NT, k == 0, False)
                    mm(ps2, b1, sil1, 0, False, True)
                # PE: ps1 of tile j+1 (tile 0 done in prologue)
                if 0 < j + 1 < T:
                    f0 = st.pop(("f0", j + 1))
                    ps1 = pp1.tile([128, NT], FP, tag="ps1")
                    st[("ps1", j + 1)] = ps1
                    mm(ps1, w0, f0, 0, True, True)

    nc.compile()
    return nc


def _host_consts(coef0, sb0, ss0, coef1, sb1, ss1, coef2, sb2, ss2):
    w0, _ = _host_weights(coef0, sb0, ss0, 0)
    w1, b1 = _host_weights(coef1, sb1, ss1, 1)
    w2, b2 = _host_weights(coef2, sb2, ss2, 2)
    gp = np.arange(96) // 12
    c0x = (((2.0 + gp) - S_BIAS) * H_GRID).astype(np.float32).reshape(96, 1)
    p = np.arange(128)
    bias = np.zeros((128, 4), np.float32)
    for j in range(4):
        gidx = 4 * (p // 64) + j
        bias[:, j] = S_BIAS - (2.0 + gidx)
    c0t = (-bias).astype(np.float32)
    return {
        "w0": w0[0], "w1": np.stack(w1), "b1": b1,
        "w2": np.stack(w2), "b2": b2, "c0x": c0x,
        "biasT": bias, "c0T": c0t,
    }


def _in_maps(x):
    consts = _CACHE["consts"]
    x = np.asarray(x, np.float32).reshape(32, 12, HW)
    maps = []
    for c in range(N_CORES):
        m = dict(consts)
        m["x_in"] = np.ascontiguousarray(x[c * B_PER_CORE:(c + 1) * B_PER_CORE])
        maps.append(m)
    return maps


def kernel(x, grid0, coef0, sb0, ss0, grid1, coef1, sb1, ss1, grid2, coef2, sb2, ss2):
    if "nc" not in _CACHE:
        _CACHE["nc"] = _build()
    nc = _CACHE["nc"]
    _CACHE["consts"] = _host_consts(
        np.asarray(coef0, np.float32), np.asarray(sb0, np.float32), np.asarray(ss0, np.float32),
        np.asarray(coef1, np.float32), np.asarray(sb1, np.float32), np.asarray(ss1, np.float32),
        np.asarray(coef2, np.float32), np.asarray(sb2, np.float32), np.asarray(ss2, np.float32))
    maps = _in_maps(x)
    res = run_bass_kernel_spmd(nc, maps, core_ids=list(range(N_CORES)))
    _CACHE["maps"] = maps
    out = np.empty((32, 24, HW), np.float32)
    for c in range(N_CORES):
        out[c * B_PER_CORE:(c + 1) * B_PER_CORE] = res.results[c]["y_out"]
    return out.reshape(32, 24, 64, 64)


def _install_ntff_hook():
    import sys, types
    if "antenv.axon_hooks" in sys.modules:
        return
    state = {"hook": None}
    mod = types.ModuleType("antenv.axon_hooks")
    mod.set_axon_ntff_profile_hook = lambda h: state.__setitem__("hook", h)
    mod.get_axon_ntff_profile_hook = lambda: state["hook"]
    sys.modules["antenv.axon_hooks"] = mod
    import antenv
    antenv.axon_hooks = mod
    from trn_agent_boot.trn_boot import _ntff_profile_via_ctypes
    hook = _ntff_profile_via_ctypes("/opt/axon/libaxon_pjrt.so")
    if hook is not None:
        mod.set_axon_ntff_profile_hook(hook)


def profile():
    _install_ntff_hook()
    nc = _CACHE["nc"]
    res = run_bass_kernel_spmd(nc, _CACHE["maps"], core_ids=list(range(N_CORES)),
                               trace=True)
    return res.exec_time_ns, getattr(res, "instructions_and_trace", None)


# revision 17
# speedup vs baseline: 1.0738x; 1.0344x over previous
"""Trainium2 Bass kernel for nn_KANCouplingNet (3-layer KAN MLP, widths 12-64-64-24).

Math: each KAN layer is y = silu(x) @ sb + B(x) contracted with coef*ss.  On
the uniform grid the basis is the cardinal cubic B-spline
    6*M(w) = relu(2-w)^3 - 4*relu(1-w)^3,   w = |s - g'|,  s = x/0.4 + 5.5.
The kernel computes the merged basis value directly (ONE feature per basis
function -> half the matmul contraction rows of the two-cube scheme) and
balances the per-page elementwise work across the Scalar and Vector engines:

  'W' pages:  Scalar: w = Abs(ps + bias_p);  u = Relu(2 - w)
              Vector: f = u^3 - 4*relu(u-1)^3          (custom op, one pass,
              batched over all W pages of a layer in a single instruction)
  'P' pages:  Vector: r2 = relu(2-|ps-c0|)^3 (paged over all P pages), then
              f = r2 - 4*relu(1-min(|ps-c0|,1))^3 per page.

Layer-0/1 stationary weights are pre-scaled by 2.5 so PSUM holds s-space
(the 5.5 shift is absorbed into the per-partition offsets); silu reads the
same PSUM with scale=0.4.  Layer 0 works in x-space directly (clamps 0.8/0.4,
weights absorb 2.5^3) on an 8-fold broadcast of x done by one DMA.

Sharding: pure data parallel over the batch dim (32 batches -> 4 per core).
"""
import dataclasses

import numpy as np

import concourse.bacc as bacc
import concourse.bass as bass
import concourse.mybir as mybir
import concourse.tile as tile
from concourse.bass_utils import run_bass_kernel_spmd

FP = mybir.dt.float32
FPR = mybir.dt.float32r
FP16 = mybir.dt.float16
AFT = mybir.ActivationFunctionType

N_CORES = 8
B_PER_CORE = 4          # 32 batches / 8 cores
HW = 64 * 64            # 4096 pixels per batch image
NT = 1024               # pixel tile
TILES_PER_B = HW // NT  # 4
H_GRID = 0.4
S_SCALE = 1.0 / H_GRID          # 2.5
S_BIAS = 2.2 / H_GRID           # 5.5
WIDTH = [12, 64, 64, 24]

# number of scalar-assisted ('W') pages per layer (first nW pages); rest 'P'
N_W = {"L1": 2, "L2": 3}

_OPS = {}
_CACHE = {}


def _register_ops():
    """Append the custom DVE ops to dve_ops.OPS (idempotent, fixed order)."""
    if _OPS:
        return _OPS
    from concourse import dve_ops
    from concourse.dve_spec import (AluOp, Bin, C0, C1, C2, One, PageIdx, Spec,
                                    Src0, Src1, Zero, _has_src1, lower, minn,
                                    relu, sq)
    from concourse.dve_uop import DveOpSpec

    def mk(name, body, ref, subdim):
        for op in dve_ops.OPS:
            if op.name == name:
                _OPS[name] = op
                return
        spec = Spec(body=body, reference=ref)
        row = dve_ops._CUSTOM_DVE_ROW_BASE + len(dve_ops.OPS)
        rd1 = _has_src1(spec)
        shas = {}
        for ver in ("v3", "v4"):
            tmp = DveOpSpec(name=name, opcode=row,
                            uops=lower(spec, ver=ver), rd1_en=rd1)
            shas[ver] = tmp.sha(ver)
        op = dve_ops.DveOp(name, spec, subdim=subdim, uops_sha=shas)
        dve_ops.OPS.append(op)
        dve_ops._SUB_OPCODE_FOR_NAME[name] = row
        dve_ops.CUSTOM_DVE_SPECS[name] = spec
        _OPS[name] = op

    # paged cube: out = relu(imm2 - |in0 - (s0 + page*s1)|)^3
    pg = PageIdx(C0, C1)
    w_pg = Bin(AluOp.ABSOLUTE_DIFF, Src0, pg)
    r_pg = relu(Bin(AluOp.SUBTRACT, C2, w_pg))

    def _ref_cube(in0, in1, s0, s1, imm2):
        in0 = np.asarray(in0, np.float32)
        if in0.ndim == 3:
            pgv = np.asarray(s0).reshape(-1, 1, 1) + np.arange(in0.shape[1]).reshape(1, -1, 1) * s1
        else:
            pgv = np.asarray(s0).reshape(-1, 1)
        r = np.maximum(imm2 - np.abs(in0 - pgv), 0.0).astype(np.float32)
        return r * r * r
    mk("CUBE_FOLD_ANT", sq(r_pg) * r_pg, _ref_cube, True)

    # W-page finish: in0 = u = relu(2-w); out = u^3 - s0*relu(u-1)^3
    u = Src0
    s1p = u - One
    R = relu(s1p)
    body_pos = (sq(u) * u) - ((s1p * R) * R) * C0

    def _ref_pos(in0, in1, s0, s1, imm2):
        u = np.asarray(in0, np.float32)
        s1p = u - 1.0
        R = np.maximum(s1p, 0.0)
        return (u * u * u - s0 * ((s1p * R) * R)).astype(np.float32)
    mk("MERGE_POS_ANT", body_pos, _ref_pos, False)

    # P-page finish: out = in1 - s1*(imm2 - min(|in0 - s0|, imm2))^3
    w = Bin(AluOp.ABSOLUTE_DIFF, Src0, C0)
    b = C2 - minn(w, C2)
    body_flat = Src1 - (sq(b) * b) * C1

    def _ref_flat(in0, in1, s0, s1, imm2):
        w = np.abs(np.asarray(in0, np.float32) - np.asarray(s0).reshape(-1, 1))
        b = imm2 - np.minimum(w, imm2)
        return (np.asarray(in1, np.float32) - s1 * b * b * b).astype(np.float32)
    mk("MERGE_FLATX_ANT", body_flat, _ref_flat, False)

    return _OPS


def _paged(ap: bass.AP, s: int) -> bass.AP:
    """View a flat [P, N] AP as [P, s, N] with a step-0 page dim."""
    return dataclasses.replace(ap, ap=[ap.ap[0], [0, s], ap.ap[1]])


def _rep8(src: bass.AP) -> bass.AP:
    """Prepend a stride-0 x8 replication dim to a DRAM [12, N] AP."""
    return dataclasses.replace(src, ap=[[0, 8]] + src.ap)


def _host_weights(coef, sb, ss, li):
    din, dout = WIDTH[li], WIDTH[li + 1]
    cp = coef.astype(np.float64) * ss.astype(np.float64)[:, :, None] / 6.0
    pre = S_SCALE if li < 2 else 1.0
    mcols = 128 if dout == 64 else dout
    if li == 0:
        scale = pre * S_SCALE ** 3       # x-space cubes absorb 2.5^3
        w = np.zeros((108, mcols), np.float32)
        for g in range(8):
            for i in range(12):
                for o in range(dout):
                    v = scale * cp[i, o, g]
                    w[g * 12 + i, o] = v
                    if mcols == 128:
                        w[g * 12 + i, o + 64] = v
        for i in range(12):
            for o in range(dout):
                w[96 + i, o] = pre * sb[i, o]
                if mcols == 128:
                    w[96 + i, o + 64] = pre * sb[i, o]
        return [w], None
    pages = []
    for j in range(4):
        w = np.zeros((128, mcols), np.float32)
        for p in range(128):
            i, g = p % 64, 4 * (p // 64) + j
            for o in range(dout):
                v = pre * cp[i, o, g]
                w[p, o] = v
                if mcols == 128:
                    w[p, o + 64] = v
        pages.append(w)
    base = np.zeros((64, mcols), np.float32)
    base[:, :dout] = pre * sb
    if mcols == 128:
        base[:, 64:64 + dout] = pre * sb
    return pages, base


def _build():
    ops = _register_ops()
    CUBE, MPOS, MFLAT = (ops["CUBE_FOLD_ANT"], ops["MERGE_POS_ANT"],
                         ops["MERGE_FLATX_ANT"])
    nc = bacc.Bacc("TRN2", target_bir_lowering=False, debug=False,
                   enable_asserts=False, num_devices=N_CORES)

    x_d = nc.dram_tensor("x_in", [B_PER_CORE, 12, HW], FP, kind="ExternalInput").ap()
    out_d = nc.dram_tensor("y_out", [B_PER_CORE, 24, HW], FP, kind="ExternalOutput").ap()
    w0_d = nc.dram_tensor("w0", [108, 128], FPR, kind="ExternalInput").ap()
    w1_d = nc.dram_tensor("w1", [4, 128, 128], FPR, kind="ExternalInput").ap()
    b1_d = nc.dram_tensor("b1", [64, 128], FPR, kind="ExternalInput").ap()
    w2_d = nc.dram_tensor("w2", [4, 128, 24], FPR, kind="ExternalInput").ap()
    b2_d = nc.dram_tensor("b2", [64, 24], FPR, kind="ExternalInput").ap()
    c0x_d = nc.dram_tensor("c0x", [96, 1], FP, kind="ExternalInput").ap()
    bias_d = nc.dram_tensor("biasT", [128, 4], FP, kind="ExternalInput").ap()
    c0t_d = nc.dram_tensor("c0T", [128, 4], FP, kind="ExternalInput").ap()

    nW1, nW2 = N_W["L1"], N_W["L2"]

    with tile.TileContext(nc) as tc:
        with (
            tc.tile_pool(name="consts", bufs=1) as cp,
            tc.tile_pool(name="xrep", bufs=3) as xp,
            tc.tile_pool(name="wabs", bufs=3) as wp,
            tc.tile_pool(name="ucl", bufs=2) as up,
            tc.tile_pool(name="r2c", bufs=2) as rp,
            tc.tile_pool(name="f0", bufs=3) as f0p,
            tc.tile_pool(name="f1", bufs=2) as f1p,
            tc.tile_pool(name="f2", bufs=2) as f2p,
            tc.tile_pool(name="sil", bufs=2) as silp,
            tc.tile_pool(name="ps1", bufs=1, space="PSUM") as pp1,
            tc.tile_pool(name="ps2", bufs=2, space="PSUM") as pp2,
            tc.tile_pool(name="ps3", bufs=2, space="PSUM") as pp3,
        ):
            # ---- constants (small per-partition vectors FIRST so the
            # feature engines can start while the weights stream in) ----
            c0x = cp.tile([96, 1], FP, tag="c0x")
            nc.sync.dma_start(c0x[:], c0x_d[:])
            biasT = cp.tile([128, 4], FP, tag="biasT")
            nc.sync.dma_start(biasT[:], bias_d[:])
            c0T = cp.tile([128, 4], FP, tag="c0T")
            nc.sync.dma_start(c0T[:], c0t_d[:])
            bias2 = cp.tile([128, 1], FP, tag="bias2")
            nc.gpsimd.memset(bias2[:], 2.0)
            w0 = cp.tile([108, 128], FPR, tag="w0")
            nc.sync.dma_start(w0[:], w0_d[:])
            b1 = cp.tile([64, 128], FPR, tag="b1")
            nc.sync.dma_start(b1[:], b1_d[:])
            b2 = cp.tile([64, 24], FPR, tag="b2")
            nc.sync.dma_start(b2[:], b2_d[:])
            w1 = [cp.tile([128, 128], FPR, tag=f"w1_{j}", name=f"w1_{j}") for j in range(4)]
            w2 = [cp.tile([128, 24], FPR, tag=f"w2_{j}", name=f"w2_{j}") for j in range(4)]
            for j in range(4):
                nc.sync.dma_start(w1[j][:], w1_d[j])
                nc.sync.dma_start(w2[j][:], w2_d[j])

            def mm(ps, w_t, f_t, col0, start, stop):
                for h in range(2):
                    nc.tensor.matmul(ps[:, h * 512:(h + 1) * 512], w_t[:],
                                     f_t[:, col0 + h * 512:col0 + (h + 1) * 512],
                                     start=start, stop=stop)

            def feat_pages(ps, f_tile, nW, u_tag, layer):
                # W pages: scalar abs+relu into u slices, one batched DVE finish
                if nW:
                    u = up.tile([128, nW * NT], FP, tag=u_tag, name=f"{u_tag}")
                    for j in range(nW):
                        wt = wp.tile([128, NT], FP, tag="wt")
                        nc.scalar.activation(wt[:], ps[:], AFT.Abs,
                                             bias=biasT[:, j:j + 1], scale=1.0)
                        nc.scalar.activation(u[:, bass.ts(j, NT)], wt[:],
                                             AFT.Relu, bias=bias2[:], scale=-1.0)
                    nc.vector._custom_dve(MPOS, out=f_tile[:, 0:nW * NT],
                                          in0=u[:], s0=4.0)
                # P pages: one paged cube + per-page flat merge
                nP = 4 - nW
                if nP:
                    r2 = rp.tile([128, nP * NT], FP, tag=f"r2_{layer}",
                                 name=f"r2_{layer}")
                    nc.vector._custom_dve(
                        CUBE, out=dataclasses.replace(
                            r2[:], ap=[r2[:].ap[0], [NT, nP], [1, NT]]),
                        in0=_paged(ps[:], nP),
                        s0=c0T[:, nW:nW + 1], s1=1.0, imm2=2.0)
                    for k in range(nP):
                        nc.vector._custom_dve(
                            MFLAT, out=f_tile[:, bass.ts(nW + k, NT)],
                            in0=ps[:], in1=r2[:, bass.ts(k, NT)],
                            s0=c0T[:, nW + k:nW + k + 1], s1=4.0, imm2=1.0)

            T = B_PER_CORE * TILES_PER_B
            st = {}

            def cols_of(t):
                b, ti = divmod(t, TILES_PER_B)
                return b, bass.ts(ti, NT)

            def dma_in(t):
                b, cols = cols_of(t)
                s0r = xp.tile([96, NT], FP, tag="s0r")
                nc.gpsimd.dma_start(s0r[:], _rep8(x_d[b, :, cols]))
                st[("s0r", t)] = s0r

            def l0_feats(t):
                s0r = st[("s0r", t)]
                f0 = f0p.tile([108, NT], FPR, tag="f0")
                r20 = rp.tile([96, NT], FP, tag="r20")
                nc.vector._custom_dve(CUBE, out=_paged(r20[:], 1),
                                      in0=_paged(s0r[:], 1),
                                      s0=c0x[:], s1=0.0, imm2=2 * H_GRID)
                nc.vector._custom_dve(MFLAT, out=f0[0:96, :], in0=s0r[:],
                                      in1=r20[:], s0=c0x[:], s1=4.0,
                                      imm2=H_GRID)
                st[("f0", t)] = f0

            def acts(t, ps, nW, u_tag):
                u = up.tile([128, nW * NT], FP, tag=u_tag, name=u_tag) if nW else None
                for j in range(nW):
                    wt = wp.tile([128, NT], FP, tag="wt")
                    nc.scalar.activation(wt[:], ps[:], AFT.Abs,
                                         bias=biasT[:, j:j + 1], scale=1.0)
                    nc.scalar.activation(u[:, bass.ts(j, NT)], wt[:],
                                         AFT.Relu, bias=bias2[:], scale=-1.0)
                return u

            def pcube(t, ps, f_tile, nW, layer):
                nP = 4 - nW
                if not nP:
                    return
                r2 = rp.tile([128, nP * NT], FP, tag=f"r2_{layer}",
                             name=f"r2_{layer}")
                nc.vector._custom_dve(
                    CUBE, out=dataclasses.replace(
                        r2[:], ap=[r2[:].ap[0], [NT, nP], [1, NT]]),
                    in0=_paged(ps[:], nP),
                    s0=c0T[:, nW:nW + 1], s1=1.0, imm2=2.0)
                for k in range(nP):
                    nc.vector._custom_dve(
                        MFLAT, out=f_tile[:, bass.ts(nW + k, NT)],
                        in0=ps[:], in1=r2[:, bass.ts(k, NT)],
                        s0=c0T[:, nW + k:nW + k + 1], s1=4.0, imm2=1.0)

            def mpos(u, f_tile, nW):
                if nW:
                    nc.vector._custom_dve(MPOS, out=f_tile[:, 0:nW * NT],
                                          in0=u[:], s0=4.0)

            dma_in(0)
            if T > 1:
                dma_in(1)
            if T > 2:
                dma_in(2)
            for t0_ in range(min(3, T)):
                l0_feats(t0_)
                s0r_ = st.pop(("s0r", t0_))
                f0_ = st[("f0", t0_)]
                nc.scalar.activation(f0_[96:108, :], s0r_[0:12, :], AFT.Silu)
            f0_ = st.pop(("f0", 0))
            ps1_ = pp1.tile([128, NT], FP, tag="ps1")
            st[("ps1", 0)] = ps1_
            mm(ps1_, w0, f0_, 0, True, True)
            for j in range(-1, T + 2):
                if 3 <= j + 2 < T:
                    dma_in(j + 2)
                # vector: L2 P-pages of tile j-1 (ps2 ready since last iter)
                if 0 <= j - 1 < T:
                    ps2 = st[("ps2", j - 1)]
                    f2 = f2p.tile([128, 4 * NT], FPR, tag="f2")
                    st[("f2", j - 1)] = f2
                    pcube(j - 1, ps2, f2, nW2, 2)
                # vector: L0 features of tile j+1 (tiles 0-2 done in prologue)
                if 3 <= j + 1 < T:
                    l0_feats(j + 1)
                # scalar: L1 activations of tile j
                if 0 <= j < T:
                    ps1 = st[("ps1", j)]
                    f1 = f1p.tile([128, 4 * NT], FPR, tag="f1")
                    st[("f1", j)] = f1
                    u1 = acts(j, ps1, nW1, "u1")
                    pcube(j, ps1, f1, nW1, 1)
                    mpos(u1, f1, nW1)
                    sil1 = silp.tile([64, NT], FPR, tag="sil1")
                    nc.scalar.activation(sil1[:], ps1[0:64, :], AFT.Silu,
                                         scale=H_GRID)
                    st[("sil1", j)] = sil1
                # scalar: L0 silu of tile j+1 (must precede its ps1 matmul)
                if 3 <= j + 1 < T:
                    s0r = st.pop(("s0r", j + 1))
                    f0 = st[("f0", j + 1)]
                    nc.scalar.activation(f0[96:108, :], s0r[0:12, :], AFT.Silu)
                # PE: ps1 of tile j+1 (tile 0 done in prologue)
                if 0 < j + 1 < T:
                    f0 = st.pop(("f0", j + 1))
                    ps1 = pp1.tile([128, NT], FP, tag="ps1")
                    st[("ps1", j + 1)] = ps1
                    mm(ps1, w0, f0, 0, True, True)
                # PE: ps2 of tile j
                if 0 <= j < T:
                    f1 = st.pop(("f1", j))
                    sil1 = st.pop(("sil1", j))
                    ps2 = pp2.tile([128, NT], FP, tag="ps2")
                    st[("ps2", j)] = ps2
                    for k in range(4):
                        mm(ps2, w1[k], f1, k * NT, k == 0, False)
                    mm(ps2, b1, sil1, 0, False, True)
                # scalar: L2 activations of tile j-1; vector: L2 W finish
                if 0 <= j - 1 < T:
                    ps2 = st.pop(("ps2", j - 1))
                    u2 = acts(j - 1, ps2, nW2, "u2")
                    f2 = st.pop(("f2", j - 1))
                    mpos(u2, f2, nW2)
                    sil2 = silp.tile([64, NT], FPR, tag="sil2")
                    nc.scalar.activation(sil2[:], ps2[0:64, :], AFT.Silu,
                                         scale=H_GRID)
                    # PE: ps3 halves + yt + out
                    halves = []
                    for h in range(2):
                        ps3 = pp3.tile([24, 512], FP, tag="ps3")
                        for k in range(4):
                            nc.tensor.matmul(ps3[:], w2[k][:],
                                             f2[:, k * NT + h * 512:k * NT + (h + 1) * 512],
                                             start=k == 0, stop=False)
                        nc.tensor.matmul(ps3[:], b2[:],
                                         sil2[:, h * 512:(h + 1) * 512],
                                         start=False, stop=True)
                        halves.append(ps3)
                    st[("ps3h", j - 1)] = halves
                # scalar tail: output copies of tile j-2
                if ("ps3h", j - 2) in st:
                    b2_, ti2 = divmod(j - 2, TILES_PER_B)
                    for h, ps3h in enumerate(st.pop(("ps3h", j - 2))):
                        yt = silp.tile([24, 512], FP, tag="yt")
                        nc.scalar.activation(yt[:], ps3h[:], AFT.Identity)
                        nc.sync.dma_start(
                            out_d[b2_, :, ti2 * NT + h * 512:ti2 * NT + (h + 1) * 512],
                            yt[:])

    nc.compile()
    return nc


def _host_consts(coef0, sb0, ss0, coef1, sb1, ss1, coef2, sb2, ss2):
    w0, _ = _host_weights(coef0, sb0, ss0, 0)
    w1, b1 = _host_weights(coef1, sb1, ss1, 1)
    w2, b2 = _host_weights(coef2, sb2, ss2, 2)
    gp = np.arange(96) // 12
    c0x = (((2.0 + gp) - S_BIAS) * H_GRID).astype(np.float32).reshape(96, 1)
    p = np.arange(128)
    bias = np.zeros((128, 4), np.float32)
    for j in range(4):
        gidx = 4 * (p // 64) + j
        bias[:, j] = S_BIAS - (2.0 + gidx)
    c0t = (-bias).astype(np.float32)
    return {
        "w0": w0[0], "w1": np.stack(w1), "b1": b1,
        "w2": np.stack(w2), "b2": b2, "c0x": c0x,
        "biasT": bias, "c0T": c0t,
    }


def _in_maps(x):
    consts = _CACHE["consts"]
    x = np.asarray(x, np.float32).reshape(32, 12, HW)
    maps = []
    for c in range(N_CORES):
        m = dict(consts)
        m["x_in"] = np.ascontiguousarray(x[c * B_PER_CORE:(c + 1) * B_PER_CORE])
        maps.append(m)
    return maps


def kernel(x, grid0, coef0, sb0, ss0, grid1, coef1, sb1, ss1, grid2, coef2, sb2, ss2):
    if "nc" not in _CACHE:
        _CACHE["nc"] = _build()
    nc = _CACHE["nc"]
    _CACHE["consts"] = _host_consts(
        np.asarray(coef0, np.float32), np.asarray(sb0, np.float32), np.asarray(ss0, np.float32),
        np.asarray(coef1, np.float32), np.asarray(sb1, np.float32), np.asarray(ss1, np.float32),
        np.asarray(coef2, np.float32), np.asarray(sb2, np.float32), np.asarray(ss2, np.float32))
    maps = _in_maps(x)
    res = run_bass_kernel_spmd(nc, maps, core_ids=list(range(N_CORES)))
    _CACHE["maps"] = maps
    out = np.empty((32, 24, HW), np.float32)
    for c in range(N_CORES):
        out[c * B_PER_CORE:(c + 1) * B_PER_CORE] = res.results[c]["y_out"]
    return out.reshape(32, 24, 64, 64)


def _install_ntff_hook():
    import sys, types
    if "antenv.axon_hooks" in sys.modules:
        return
    state = {"hook": None}
    mod = types.ModuleType("antenv.axon_hooks")
    mod.set_axon_ntff_profile_hook = lambda h: state.__setitem__("hook", h)
    mod.get_axon_ntff_profile_hook = lambda: state["hook"]
    sys.modules["antenv.axon_hooks"] = mod
    import antenv
    antenv.axon_hooks = mod
    from trn_agent_boot.trn_boot import _ntff_profile_via_ctypes
    hook = _ntff_profile_via_ctypes("/opt/axon/libaxon_pjrt.so")
    if hook is not None:
        mod.set_axon_ntff_profile_hook(hook)


def profile():
    _install_ntff_hook()
    nc = _CACHE["nc"]
    res = run_bass_kernel_spmd(nc, _CACHE["maps"], core_ids=list(range(N_CORES)),
                               trace=True)
    return res.exec_time_ns, getattr(res, "instructions_and_trace", None)
